# revision 26
# baseline (speedup 1.0000x reference)
"""Trainium2 Bass kernel: CumulativeSetFeatures (histogram binning).

Computes, from X[16,256,4096], projections[100,256], min/max[100]:
  a[b,p,l]   = sum_c X[b,c,l] * proj[p,c]           (PE matmul, fp32)
  thr[p,q]   = min[p] + (max[p]-min[p]) * (q+1)/21  (host, tiny)
  set[b,p*20+q,l] = (a[b,p,l] < thr[p,q])           (DVE/Pool tensor_scalar is_lt)
  cdf[b,p*20+q]   = mean_l set                      (accum_out / 4096)

Sharding: data-parallel over batch B=16 across 8 cores (2 batches/core);
projections + thresholds replicated. Output gathered by concatenation.

Layout/perf choices:
- set values are exactly 0.0/1.0 -> stored as uint8 on device and upcast
  on the host (exact); quarters the dominant HBM write traffic.
- 4 consecutive q-rows (adjacent output rows) are packed per store DMA so
  each partition writes long contiguous runs.
- X loads (l-halves), matmuls (512-chunks) and compares (l-quarters) are
  software-pipelined; compare work is split ~73/27 between DVE and Pool
  (cost-balanced), ACT accumulates cdf columns for Pool's compares (Pool
  lacks the fused accum variant) and drains PSUM.
- 6 store buffers so batch 1's compares never stall on batch 0's drain.
"""

import numpy as np

import concourse.bacc as bacc
import concourse.bass as bass
import concourse.mybir as mybir
import concourse.tile as tile
from concourse.bass_utils import run_bass_kernel_spmd

B, C, L = 16, 256, 4096
P, Q = 100, 20
N_CORES = 8
B_LOC = B // N_CORES          # batches per core
KP = 128                      # contraction partition tile
KC = C // KP                  # 2 k-chunks
LT = 512                      # l-chunk = one PSUM bank of fp32
NH = 4                        # l-quarters for compare pipelining
LH = L // NH
NHS = 2                       # l-halves for X loads and store DMAs
LHS = L // NHS
PQ = P * Q
GQ = 4                        # q's per grouped store DMA
NG = Q // GQ
F32 = mybir.dt.float32

SET_DT = mybir.dt.uint8       # on-device set storage dtype (exact for 0/1)
POOL_Q18 = False

_cached_nc = None


def _on_pool(q, h, b=0):
    # Pool takes ~27% of compare ops (matching the DVE:Pool per-op cost
    # ratio), spread across store groups so ACT's follow-up cdf
    # accumulations never pile up behind a trailing burst of Pool work;
    # one late-group half is shifted to Pool to even out finish times.
    if POOL_Q18 and b == 1 and q == 18 and h < NH // 2:
        return True
    return q % 4 == 1 or (q == 2 and h < NH // 2)


def _build():
    nc = bacc.Bacc(
        "TRN2",
        target_bir_lowering=False,
        debug=False,
        enable_asserts=False,
        num_devices=N_CORES,
    )
    X_d = nc.dram_tensor("X", [B_LOC, C, L], F32, kind="ExternalInput")
    projT_d = nc.dram_tensor("projT", [C, P], F32, kind="ExternalInput")
    thr_d = nc.dram_tensor("thr", [P, Q], F32, kind="ExternalInput")
    set_d = nc.dram_tensor("set_out", [B_LOC, PQ, L], SET_DT, kind="ExternalOutput")
    cdf_d = nc.dram_tensor("cdf_out", [B_LOC, PQ], F32, kind="ExternalOutput")

    with tile.TileContext(nc) as tc:
        with (
            tc.tile_pool(name="singles", bufs=1) as singles,
            tc.tile_pool(name="xpool", bufs=3) as xpool,
            tc.tile_pool(name="apool", bufs=2) as apool,
            tc.tile_pool(name="setp", bufs=6) as setp,
            tc.tile_pool(name="cdfp", bufs=2) as cdfp,
            tc.tile_pool(name="psum", bufs=4, space=bass.MemorySpace.PSUM) as psum,
        ):
            # constants go through Pool's SWDGE ring so the X loads own the
            # HWDGE ring from t=0
            projT_s = singles.tile([KP, KC, P], F32)
            nc.gpsimd.dma_start(
                out=projT_s, in_=projT_d.ap().rearrange("(kc k) m -> k kc m", k=KP)
            )
            thr_s = singles.tile([P, Q], F32)
            nc.gpsimd.dma_start(out=thr_s, in_=thr_d.ap())
            act_scratch = singles.tile([P, LHS], SET_DT)

            # Dummy matmuls on a zeroed scratch tile warm the PE HAM clock
            # gate (1.2 -> 2.4 GHz) while the first X chunks are still in
            # flight, so the real matmul stream runs warm from the start.
            warm_s = singles.tile([KP, LT], F32)
            nc.vector.memset(warm_s, 0.0)
            warm_ps = psum.tile([P, LT], F32)
            for _ in range(2):
                nc.tensor.matmul(
                    warm_ps,
                    warm_s[:, :P],
                    warm_s[:, :],
                    start=True,
                    stop=True,
                )

            # [b, p, g, j, l]: group g covers q = 4g..4g+3, adjacent output
            # rows -> one DMA writes long contiguous runs per partition.
            set_r = set_d.ap().rearrange("b (p g j) l -> b p g j l", g=NG, j=GQ)
            cdf_r = cdf_d.ap().rearrange("b (p q) -> b p q", q=Q)
            x_r = X_d.ap().rearrange("b (kc k) l -> b k kc l", k=KP)

            def load_x(b, hs_list):
                # X arrives as half-of-L tiles, each holding both k-chunks;
                # 1MB sub-loads, kc-interleaved, so the first matmul can start
                # as soon as the first quarter of both k-chunks lands
                out = []
                for hs in hs_list:
                    xt = xpool.tile([KP, KC, LHS], F32, tag="x")
                    for sub in range(2):
                        for kc in range(KC):
                            lo = hs * LHS + sub * (LHS // 2)
                            nc.scalar.dma_start(
                                out=xt[:, kc, sub * (LHS // 2) : (sub + 1) * (LHS // 2)],
                                in_=x_r[b, :, kc, lo : lo + LHS // 2],
                            )
                    out.append(xt)
                return out

            x_next = None
            for b in range(B_LOC):
                x_h = load_x(b, range(NHS)) if x_next is None else x_next
                a_s = apool.tile([P, L], F32)
                for lc in range(L // LT):
                    xt = x_h[lc * LT // LHS]
                    xo = (lc * LT) % LHS
                    ps = psum.tile([P, LT], F32)
                    nc.tensor.matmul(
                        ps,
                        projT_s[:, 0, :],
                        xt[:, 0, xo : xo + LT],
                        start=True,
                        stop=False,
                    )
                    nc.tensor.matmul(
                        ps,
                        projT_s[:, 1, :],
                        xt[:, 1, xo : xo + LT],
                        start=False,
                        stop=True,
                    )
                    nc.scalar.copy(out=a_s[:, lc * LT : (lc + 1) * LT], in_=ps[:, :])

                # Disjoint accumulator tiles per producing engine (avoids
                # cross-engine serialization on a shared tile); per-l-quarter
                # partial counts are summed at the end (exact integers).
                cdf_v = cdfp.tile([P, Q, NH + 1], F32)
                cdf_a = cdfp.tile([P, Q, NH + 1], F32)
                nc.vector.memset(cdf_v, 0.0)
                nc.scalar.memzero(cdf_a)
                for g in range(NG):
                    set_s = setp.tile([P, GQ, L], SET_DT)
                    for hs in range(NHS):
                        # quarter-granular streaming (a_s is produced
                        # just-in-time for both batches); the very first chunk
                        # is split to 512 cols so compares start right after
                        # the first PSUM drains.
                        pieces = []
                        for h in range(hs * NH // NHS, (hs + 1) * NH // NHS):
                            if b == 0 and g == 0 and h == 0:
                                pieces += [(slice(0, LT), NH, h),
                                           (slice(LT, LH), h, h)]
                            else:
                                pieces += [
                                    (slice(h * LH, (h + 1) * LH), h, h)
                                ]
                        for hl, hslot, h in pieces:
                                for j in range(GQ):
                                    q = g * GQ + j
                                    if _on_pool(q, h, b):
                                        # Pool codegen rejects the fused accum
                                        # variant: bare compare, ACT accums below.
                                        nc.gpsimd.tensor_scalar(
                                            out=set_s[:, j, hl],
                                            in0=a_s[:, hl],
                                            scalar1=thr_s[:, q : q + 1],
                                            scalar2=None,
                                            op0=mybir.AluOpType.is_lt,
                                        )
                                    else:
                                        nc.vector.tensor_scalar(
                                            out=set_s[:, j, hl],
                                            in0=a_s[:, hl],
                                            scalar1=thr_s[:, q : q + 1],
                                            scalar2=0.0,
                                            op0=mybir.AluOpType.is_lt,
                                            op1=mybir.AluOpType.add,
                                            accum_out=cdf_v[:, q, hslot : hslot + 1],
                                        )
                        # cdf columns for Pool's compares: one ACT accum per
                        # l-half (cheaper than per-quarter)
                        sls = slice(hs * LHS, (hs + 1) * LHS)
                        for j in range(GQ):
                            q = g * GQ + j
                            if _on_pool(q, hs * NH // NHS, b) or _on_pool(
                                q, (hs + 1) * NH // NHS - 1, b
                            ):
                                nc.scalar.activation(
                                    out=act_scratch[:, :],
                                    in_=set_s[:, j, sls],
                                    func=mybir.ActivationFunctionType.Copy,
                                    accum_out=cdf_a[:, q, hs : hs + 1],
                                )
                        # store this l-half of the 4-row group (8KB runs)
                        nc.sync.dma_start(
                            out=set_r[b, :, g, :, sls], in_=set_s[:, :, sls]
                        )
                    # stage the next batch's X loads mid-way through this
                    # batch's compare groups: early enough to keep PE fed,
                    # late enough not to delay this batch's stores
                    if b + 1 < B_LOC and g == 1:
                        x_next = load_x(b + 1, range(NHS))

                cdfo_s = cdfp.tile([P, Q], F32)
                cdfo_t = cdfp.tile([P, Q, NH + 1], F32)
                nc.vector.tensor_tensor(
                    out=cdfo_t[:, :, :],
                    in0=cdf_v[:, :, :],
                    in1=cdf_a[:, :, :],
                    op=mybir.AluOpType.add,
                )
                nc.vector.tensor_reduce(
                    out=cdfo_s[:, :],
                    in_=cdfo_t[:, :, :],
                    axis=mybir.AxisListType.X,
                    op=mybir.AluOpType.add,
                )
                nc.scalar.mul(out=cdfo_s[:, :], in_=cdfo_s[:, :], mul=1.0 / L)
                nc.scalar.dma_start(out=cdf_r[b], in_=cdfo_s[:, :])

    nc.compile()
    return nc


def kernel(X, projections, min_vals, max_vals):
    global _cached_nc
    X = np.ascontiguousarray(np.asarray(X, dtype=np.float32))
    projections = np.asarray(projections, dtype=np.float32)
    min_vals = np.asarray(min_vals, dtype=np.float32)
    max_vals = np.asarray(max_vals, dtype=np.float32)

    if _cached_nc is None:
        _cached_nc = _build()
    nc = _cached_nc

    projT = np.ascontiguousarray(projections.T)
    fracs = np.arange(1, Q + 1, dtype=np.float32) / np.float32(Q + 1)
    thr = min_vals[:, None] + (max_vals - min_vals)[:, None] * fracs[None, :]
    thr = np.ascontiguousarray(thr.astype(np.float32))

    in_maps = [
        {"X": X[c * B_LOC : (c + 1) * B_LOC], "projT": projT, "thr": thr}
        for c in range(N_CORES)
    ]
    res = run_bass_kernel_spmd(nc, in_maps, core_ids=list(range(N_CORES)))
    cdf_out = np.concatenate([r["cdf_out"] for r in res.results], axis=0)
    set_out = np.concatenate([r["set_out"] for r in res.results], axis=0)
    if set_out.dtype != np.float32:
        set_out = set_out.astype(np.float32)
    return cdf_out, set_out
